# revision 34
# baseline (speedup 1.0000x reference)
"""CoupledLSTM Trainium2 kernel.

Problem: S=512, B=64, I=H=512 coupled-gate LSTM (f = 1-i), fp32 reference.

Strategy (8 NeuronCores, data-parallel over batch, 8 batch rows per core):
  - All device-side tensors keep hidden on the partition dim ("transposed"
    layout); the host does every layout transpose in numpy for free.
  - Phase A: xg[g] = x @ W_x[g].T + b[g] for all (t, b) as big matmuls
    (fp16 in, fp32 accumulate), kept SBUF-resident in fp16.
  - Phase B: 512 sequential steps; per step 48 [128x128]@[128x8] fp16
    matmuls (weight-port bound), fp32 elementwise on [128, 32] tiles.
Gate order everywhere: (c, i, o) so the c-gate chain overlaps i/o matmuls.
"""

import numpy as np

S, B, I, H = 512, 64, 512, 512
NCORES = 8
BL = B // NCORES  # 8 local batch rows
QI = I // 128  # 4 input chunks
QH = H // 128  # 4 hidden chunks
NB = (S * BL) // 512  # 8 phase-A N-blocks of 512 (t,b) columns
STEPS = S

_CACHE = {}


def _build_nc(steps=STEPS):
    import concourse.bacc as bacc
    import concourse.mybir as mybir
    import concourse.tile as tile

    f32 = mybir.dt.float32
    f16 = mybir.dt.float16
    AF = mybir.ActivationFunctionType

    nc = bacc.Bacc("TRN2", target_bir_lowering=False, debug=False, num_devices=NCORES)

    xT_d = nc.dram_tensor("xT", [I, S * BL], f16, kind="ExternalInput").ap()
    WxT_d = nc.dram_tensor("WxT", [3 * QH * QI * 128, 128], f16, kind="ExternalInput").ap()
    WhT_d = nc.dram_tensor("WhT", [3 * QH * QH * 128, 128], f16, kind="ExternalInput").ap()
    bias_d = nc.dram_tensor("bias", [128, 3 * QH], f32, kind="ExternalInput").ap()
    ident_d = nc.dram_tensor("ident", [128, 128], f16, kind="ExternalInput").ap()
    outT_d = nc.dram_tensor("outT", [S, 128, QH * BL], f16, kind="ExternalOutput").ap()
    cT_d = nc.dram_tensor("cT", [128, QH * BL], f32, kind="ExternalOutput").ap()

    GW = QH * BL  # 32: per-gate slab width (q, b)
    TIB = 512 // BL  # 64 steps per nb block
    with tile.TileContext(nc) as tc:
        with tc.tile_pool(name="persist", bufs=1) as persist, tc.tile_pool(
            name="state", bufs=2
        ) as st, tc.tile_pool(name="work", bufs=3) as wk, tc.tile_pool(
            name="b_psum", bufs=2, space="PSUM"
        ) as pbp, tc.tile_pool(name="pa_psum", bufs=2, space="PSUM") as pap:
            # xg_all[p, nb, gq, ti, b] fp16 (gate order c,i,o; gq = g*QH+qo)
            xg_all = persist.tile([128, NB, 12, TIB, BL], f16)
            WhT = persist.tile([128, 3 * QH * QH * 128], f16)
            WxT = persist.tile([128, 3 * QH * QI * 128], f16)
            bias = persist.tile([128, 3 * QH], f32)
            nc.sync.dma_start(bias[:], bias_d[:])
            ident = persist.tile([128, 128], f16)
            nc.sync.dma_start(ident[:], ident_d[:])
            # W dram rows: tile tt=(g,qo,qi) at rows [tt*128,(tt+1)*128);
            # SBUF wants [p_in, tt*128 + p_out]. Phase-A inputs (WxT, x)
            # load first so projection matmuls start ASAP; WhT (needed at
            # step 0, ~30us later) trails on the gpsimd queue.
            WhT_src = WhT_d.rearrange("(t p) m -> p t m", p=128)
            WxT_src = WxT_d.rearrange("(t p) m -> p t m", p=128)
            qs = [nc.sync, nc.gpsimd, nc.scalar]
            for i4 in range(4):
                qs[i4 % 2].dma_start(
                    WxT[:, i4 * 12 * 128 : (i4 + 1) * 12 * 128],
                    WxT_src[:, i4 * 12 : (i4 + 1) * 12, :],
                )
            xts = []
            for qi in range(QI):
                xt = persist.tile([128, S * BL], f16, tag=f"xt{qi}")
                qs[2 if qi % 2 else 0].dma_start(xt[:], xT_d[qi * 128 : (qi + 1) * 128, :])
                xts.append(xt)
            for i4 in range(4):
                qs[i4 % 2].dma_start(
                    WhT[:, i4 * 12 * 128 : (i4 + 1) * 12 * 128],
                    WhT_src[:, i4 * 12 : (i4 + 1) * 12, :],
                )

            def pa_item(nb, gjc):
                # xg[nb, gjc] = W_x tile row @ x block + bias (one psum bank)
                ps = pap.tile([128, 512], f32, tag="pa")
                for qi in range(QI):
                    tt = gjc * QI + qi
                    nc.tensor.matmul(
                        ps[:],
                        WxT[:, tt * 128 : (tt + 1) * 128],
                        xts[qi][:, nb * 512 : (nb + 1) * 512],
                        start=(qi == 0),
                        stop=(qi == QI - 1),
                    )
                nc.vector.tensor_scalar_add(
                    out=xg_all[:, nb, gjc, :, :],
                    in0=ps[:],
                    scalar1=bias[:, gjc : gjc + 1],
                )

            def pa_mms(nb, gjc, t_lo=0, t_hi=TIB):
                ps = pap.tile([128, 512], f32, tag="pa")
                w = (t_hi - t_lo) * BL
                for qi in range(QI):
                    tt = gjc * QI + qi
                    nc.tensor.matmul(
                        ps[:, :w],
                        WxT[:, tt * 128 : (tt + 1) * 128],
                        xts[qi][:, nb * 512 + t_lo * BL : nb * 512 + t_hi * BL],
                        start=(qi == 0),
                        stop=(qi == QI - 1),
                    )
                return ps

            def pa_evac(ps, nb, gjc, t_lo=0, t_hi=TIB):
                nc.vector.tensor_scalar_add(
                    out=xg_all[:, nb, gjc, t_lo:t_hi, :],
                    in0=ps[:, : (t_hi - t_lo) * BL],
                    scalar1=bias[:, gjc : gjc + 1],
                )

            # phase A upfront: first nb block (steps 0..63); the rest is
            # dripped into the PE-idle tail windows one block ahead of use.
            # A dripped item's matmuls land at the end of step s, its psum
            # evacuation two steps later, so the evac never crowds the
            # critical DVE chain.
            # Upfront: all of nb=0; nb>=1 items drip one block ahead, one
            # item per 5 steps (denser spacing measured slower). Drip keys:
            # (nb, gjc, t_lo, t_hi).
            for gjc in range(12):
                ps = pa_mms(0, gjc)
                pa_evac(ps, 0, gjc)
            drip_mm, drip_ev = {}, {}
            for nb in range(1, NB):
                base = (nb - 1) * TIB + 2
                for k in range(12):
                    it = (nb, k, 0, TIB)
                    drip_mm.setdefault(base + k * 5, []).append(it)
                    drip_ev.setdefault(base + k * 5 + 2, []).append(it)
            drip_ps = {}

            h16 = st.tile([128, GW], f16, tag="h16")
            nc.vector.memset(h16[:], 0.0)
            c_st = st.tile([128, GW], f32, tag="c")
            nc.vector.memset(c_st[:], 0.0)

            def gate_mm(g, out_ap, h_prev, xg_slab):
                # out = W_hg @ h + xg_g (identity matmul adds xg into
                # psum). One PSUM bank allows a single start/stop
                # accumulation group: the identity matmul opens it
                # (start=True zeroes the bank), the last W-matmul closes.
                nc.tensor.matmul(out_ap[:], ident[:], xg_slab, start=True, stop=False)
                for qo in range(QH):
                    for qi in range(QH):
                        tt = (g * QH + qo) * QH + qi
                        nc.tensor.matmul(
                            out_ap[:, qo * BL : (qo + 1) * BL],
                            WhT[:, tt * 128 : (tt + 1) * 128],
                            h_prev[:, qi * BL : (qi + 1) * BL],
                            start=False,
                            stop=(qo == QH - 1 and qi == QH - 1),
                        )

            for t in range(steps):
                nb, ti = divmod(t, TIB)

                # c and i gates share one PSUM bank: one identity matmul
                # opens the group for both; chat still fires as soon as the
                # c-range writers land (range-based deps, not group stop).
                ps_ci = pbp.tile([128, 2 * GW], f32, tag="ps_ci")
                ps_c = ps_ci[:, :GW]
                ps_i = ps_ci[:, GW:]
                nc.tensor.matmul(
                    ps_ci[:], ident[:], xg_all[:, nb, 0 : 2 * QH, ti, :],
                    start=True, stop=False,
                )
                for g, base in ((0, 0), (1, GW)):
                    for qo in range(QH):
                        for qi in range(QH):
                            tt = (g * QH + qo) * QH + qi
                            nc.tensor.matmul(
                                ps_ci[:, base + qo * BL : base + (qo + 1) * BL],
                                WhT[:, tt * 128 : (tt + 1) * 128],
                                h16[:, qi * BL : (qi + 1) * BL],
                                start=False,
                                stop=(g == 1 and qo == QH - 1 and qi == QH - 1),
                            )
                ps_o = pbp.tile([128, GW], f32, tag="ps_o")
                gate_mm(2, ps_o[:], h16, xg_all[:, nb, 2 * QH :, ti, :])

                chat = wk.tile([128, GW], f32, tag="chat")
                nc.scalar.activation(chat[:], ps_c[:], AF.Tanh)
                d_t = wk.tile([128, GW], f32, tag="d")
                nc.vector.tensor_sub(d_t[:], chat[:], c_st[:])

                i_t = wk.tile([128, GW], f32, tag="i")
                nc.scalar.activation(i_t[:], ps_i[:], AF.Sigmoid)
                e_t = wk.tile([128, GW], f32, tag="e")
                nc.vector.tensor_mul(e_t[:], i_t[:], d_t[:])
                c_new = st.tile([128, GW], f32, tag="c")
                nc.vector.tensor_add(c_new[:], c_st[:], e_t[:])
                th = wk.tile([128, GW], f32, tag="th")
                nc.scalar.activation(th[:], c_new[:], AF.Tanh)

                o_t = wk.tile([128, GW], f32, tag="o")
                nc.scalar.activation(o_t[:], ps_o[:], AF.Sigmoid)
                h_new = st.tile([128, GW], f16, tag="h16")
                nc.vector.tensor_mul(h_new[:], o_t[:], th[:])
                nc.sync.dma_start(outT_d[t], h_new[:])

                for it in drip_mm.get(t, ()):
                    drip_ps[it] = pa_mms(*it)
                for it in drip_ev.get(t, ()):
                    pa_evac(drip_ps.pop(it), *it)

                h16 = h_new
                c_st = c_new

            nc.sync.dma_start(cT_d[:], c_st[:])

    nc.finalize()
    return nc


def _prep_weights(W_list):
    # W [H, K] -> tiles [(g qo qi) p_in, p_out]
    out = []
    for W in W_list:
        Wt = np.asarray(W, np.float32).reshape(QH, 128, -1, 128)  # qo p_out qi p_in
        out.append(np.transpose(Wt, (0, 2, 3, 1)))  # qo qi p_in p_out
    arr = np.stack(out, 0)  # g qo qi p_in p_out
    return np.ascontiguousarray(arr.reshape(-1, 128)).astype(np.float16)


def kernel(x, W_xi, W_hi, b_i, W_xc, W_hc, b_c, W_xo, W_ho, b_o):
    x = np.asarray(x, np.float32)
    # gate order (c, i, o)
    WxT = _prep_weights([W_xc, W_xi, W_xo])
    WhT = _prep_weights([W_hc, W_hi, W_ho])
    bias = np.stack(
        [np.asarray(b, np.float32).reshape(QH, 128).T for b in (b_c, b_i, b_o)], 1
    ).reshape(128, 3 * QH)

    if "nc" not in _CACHE:
        _CACHE["nc"] = _build_nc()
    nc = _CACHE["nc"]

    in_maps = []
    for c in range(NCORES):
        xs = x[:, c * BL : (c + 1) * BL, :]  # [S, BL, I]
        xT = np.ascontiguousarray(np.transpose(xs, (2, 0, 1)).reshape(I, S * BL))
        in_maps.append(
            {
                "xT": xT.astype(np.float16),
                "WxT": WxT,
                "WhT": WhT,
                "bias": bias,
                "ident": np.eye(128, dtype=np.float16),
            }
        )

    from concourse.bass_utils import run_bass_kernel_spmd

    _CACHE["in_maps"] = in_maps
    res = run_bass_kernel_spmd(nc, in_maps, list(range(NCORES)))

    output = np.empty((S, B, H), np.float32)
    c_fin = np.empty((B, H), np.float32)
    for c in range(NCORES):
        oT = res.results[c]["outT"].astype(np.float32)  # [S, 128, QH*BL]
        output[:, c * BL : (c + 1) * BL, :] = (
            oT.reshape(S, 128, QH, BL).transpose(0, 3, 2, 1).reshape(S, BL, H)
        )
        cT = res.results[c]["cT"]
        c_fin[c * BL : (c + 1) * BL] = (
            cT.reshape(128, QH, BL).transpose(2, 1, 0).reshape(BL, H)
        )
    h_fin = np.ascontiguousarray(output[-1])
    return output, h_fin, c_fin


# revision 35
# speedup vs baseline: 1.2251x; 1.2251x over previous
"""CoupledLSTM Trainium2 kernel.

Problem: S=512, B=64, I=H=512 coupled-gate LSTM (f = 1-i), fp32 reference.

Strategy (8 NeuronCores, data-parallel over batch, 8 batch rows per core):
  - All device-side tensors keep hidden on the partition dim ("transposed"
    layout); the host does every layout transpose in numpy for free.
  - Phase A: xg[g] = x @ W_x[g].T + b[g] for all (t, b) as big matmuls
    (fp16 in, fp32 accumulate), kept SBUF-resident in fp16.
  - Phase B: 512 sequential steps; per step 48 [128x128]@[128x8] fp16
    matmuls (weight-port bound), fp32 elementwise on [128, 32] tiles.
Gate order everywhere: (c, i, o) so the c-gate chain overlaps i/o matmuls.
"""

import numpy as np

S, B, I, H = 512, 64, 512, 512
NCORES = 8
BL = B // NCORES  # 8 local batch rows
QI = I // 128  # 4 input chunks
QH = H // 128  # 4 hidden chunks
NB = (S * BL) // 512  # 8 phase-A N-blocks of 512 (t,b) columns
STEPS = S

_CACHE = {}


def _build_nc(steps=STEPS):
    import concourse.bacc as bacc
    import concourse.mybir as mybir
    import concourse.tile as tile

    f32 = mybir.dt.float32
    f16 = mybir.dt.float16
    AF = mybir.ActivationFunctionType

    nc = bacc.Bacc("TRN2", target_bir_lowering=False, debug=False, num_devices=NCORES)

    xT_d = nc.dram_tensor("xT", [I, S * BL], f16, kind="ExternalInput").ap()
    WxT_d = nc.dram_tensor("WxT", [3 * QH * QI * 128, 128], f16, kind="ExternalInput").ap()
    WhT_d = nc.dram_tensor("WhT", [3 * QH * QH * 128, 128], f16, kind="ExternalInput").ap()
    bias_d = nc.dram_tensor("bias", [128, 3 * QH], f32, kind="ExternalInput").ap()
    ident_d = nc.dram_tensor("ident", [128, 128], f16, kind="ExternalInput").ap()
    outT_d = nc.dram_tensor("outT", [S, 128, QH * BL], f16, kind="ExternalOutput").ap()
    cT_d = nc.dram_tensor("cT", [128, QH * BL], f32, kind="ExternalOutput").ap()

    GW = QH * BL  # 32: per-gate slab width (q, b)
    TIB = 512 // BL  # 64 steps per nb block
    with tile.TileContext(nc) as tc:
        with tc.tile_pool(name="persist", bufs=1) as persist, tc.tile_pool(
            name="state", bufs=2
        ) as st, tc.tile_pool(name="work", bufs=3) as wk, tc.tile_pool(
            name="b_psum", bufs=2, space="PSUM"
        ) as pbp, tc.tile_pool(name="pa_psum", bufs=2, space="PSUM") as pap:
            # xg_all[p, nb, gq, ti, b] fp16 (gate order c,i,o; gq = g*QH+qo)
            xg_all = persist.tile([128, NB, 12, TIB, BL], f16)
            WhT = persist.tile([128, 3 * QH * QH * 128], f16)
            WxT = persist.tile([128, 3 * QH * QI * 128], f16)
            bias = persist.tile([128, 3 * QH], f32)
            nc.sync.dma_start(bias[:], bias_d[:])
            ident = persist.tile([128, 128], f16)
            nc.sync.dma_start(ident[:], ident_d[:])
            # W dram rows: tile tt=(g,qo,qi) at rows [tt*128,(tt+1)*128);
            # SBUF wants [p_in, tt*128 + p_out]. Phase-A inputs (WxT, x)
            # load first so projection matmuls start ASAP; WhT (needed at
            # step 0, ~30us later) trails on the gpsimd queue.
            WhT_src = WhT_d.rearrange("(t p) m -> p t m", p=128)
            WxT_src = WxT_d.rearrange("(t p) m -> p t m", p=128)
            qs = [nc.sync, nc.gpsimd, nc.scalar]
            for i4 in range(4):
                qs[i4 % 2].dma_start(
                    WxT[:, i4 * 12 * 128 : (i4 + 1) * 12 * 128],
                    WxT_src[:, i4 * 12 : (i4 + 1) * 12, :],
                )
            xts = []
            for qi in range(QI):
                xt = persist.tile([128, S * BL], f16, tag=f"xt{qi}")
                qs[2 if qi % 2 else 0].dma_start(xt[:], xT_d[qi * 128 : (qi + 1) * 128, :])
                xts.append(xt)
            for i4 in range(4):
                qs[i4 % 2].dma_start(
                    WhT[:, i4 * 12 * 128 : (i4 + 1) * 12 * 128],
                    WhT_src[:, i4 * 12 : (i4 + 1) * 12, :],
                )

            def pa_item(nb, gjc):
                # xg[nb, gjc] = W_x tile row @ x block + bias (one psum bank)
                ps = pap.tile([128, 512], f32, tag="pa")
                for qi in range(QI):
                    tt = gjc * QI + qi
                    nc.tensor.matmul(
                        ps[:],
                        WxT[:, tt * 128 : (tt + 1) * 128],
                        xts[qi][:, nb * 512 : (nb + 1) * 512],
                        start=(qi == 0),
                        stop=(qi == QI - 1),
                    )
                nc.vector.tensor_scalar_add(
                    out=xg_all[:, nb, gjc, :, :],
                    in0=ps[:],
                    scalar1=bias[:, gjc : gjc + 1],
                )

            def pa_mms(nb, gjc, t_lo=0, t_hi=TIB):
                ps = pap.tile([128, 512], f32, tag="pa")
                w = (t_hi - t_lo) * BL
                for qi in range(QI):
                    tt = gjc * QI + qi
                    nc.tensor.matmul(
                        ps[:, :w],
                        WxT[:, tt * 128 : (tt + 1) * 128],
                        xts[qi][:, nb * 512 + t_lo * BL : nb * 512 + t_hi * BL],
                        start=(qi == 0),
                        stop=(qi == QI - 1),
                    )
                return ps

            def pa_evac(ps, nb, gjc, t_lo=0, t_hi=TIB):
                nc.vector.tensor_scalar_add(
                    out=xg_all[:, nb, gjc, t_lo:t_hi, :],
                    in0=ps[:, : (t_hi - t_lo) * BL],
                    scalar1=bias[:, gjc : gjc + 1],
                )

            # phase A upfront: first nb block (steps 0..63); the rest is
            # dripped into the PE-idle tail windows one block ahead of use.
            # A dripped item's matmuls land at the end of step s, its psum
            # evacuation two steps later, so the evac never crowds the
            # critical DVE chain.
            # Upfront: all of nb=0; nb>=1 items drip one block ahead, one
            # item per 5 steps (denser spacing measured slower). Drip keys:
            # (nb, gjc, t_lo, t_hi).
            for gjc in range(12):
                ps = pa_mms(0, gjc)
                pa_evac(ps, 0, gjc)
            drip_mm, drip_ev = {}, {}
            for nb in range(1, NB):
                base = (nb - 1) * TIB + 2
                for k in range(12):
                    it = (nb, k, 0, TIB)
                    drip_mm.setdefault(base + k * 5, []).append(it)
                    drip_ev.setdefault(base + k * 5 + 2, []).append(it)
            drip_ps = {}

            h16 = st.tile([128, GW], f16, tag="h16")
            nc.vector.memset(h16[:], 0.0)
            c_st = st.tile([128, GW], f32, tag="c")
            nc.vector.memset(c_st[:], 0.0)

            def gate_mm(g, out_ap, h_prev, xg_slab):
                # out = W_hg @ h + xg_g (identity matmul adds xg into
                # psum). One PSUM bank allows a single start/stop
                # accumulation group: the identity matmul opens it
                # (start=True zeroes the bank), the last W-matmul closes.
                nc.tensor.matmul(out_ap[:], ident[:], xg_slab, start=True, stop=False)
                for qo in range(QH):
                    for qi in range(QH):
                        tt = (g * QH + qo) * QH + qi
                        nc.tensor.matmul(
                            out_ap[:, qo * BL : (qo + 1) * BL],
                            WhT[:, tt * 128 : (tt + 1) * 128],
                            h_prev[:, qi * BL : (qi + 1) * BL],
                            start=False,
                            stop=(qo == QH - 1 and qi == QH - 1),
                        )

            for t in range(steps):
                nb, ti = divmod(t, TIB)

                ps_c = pbp.tile([128, GW], f32, tag="ps_c")
                gate_mm(0, ps_c[:], h16, xg_all[:, nb, 0:QH, ti, :])
                ps_i = pbp.tile([128, GW], f32, tag="ps_i")
                gate_mm(1, ps_i[:], h16, xg_all[:, nb, QH : 2 * QH, ti, :])
                ps_o = pbp.tile([128, GW], f32, tag="ps_o")
                gate_mm(2, ps_o[:], h16, xg_all[:, nb, 2 * QH :, ti, :])

                chat = wk.tile([128, GW], f32, tag="chat")
                nc.scalar.activation(chat[:], ps_c[:], AF.Tanh)
                d_t = wk.tile([128, GW], f32, tag="d")
                nc.vector.tensor_sub(d_t[:], chat[:], c_st[:])

                i_t = wk.tile([128, GW], f32, tag="i")
                nc.scalar.activation(i_t[:], ps_i[:], AF.Sigmoid)
                e_t = wk.tile([128, GW], f32, tag="e")
                nc.vector.tensor_mul(e_t[:], i_t[:], d_t[:])
                c_new = st.tile([128, GW], f32, tag="c")
                nc.vector.tensor_add(c_new[:], c_st[:], e_t[:])
                th = wk.tile([128, GW], f32, tag="th")
                nc.scalar.activation(th[:], c_new[:], AF.Tanh)

                o_t = wk.tile([128, GW], f32, tag="o")
                nc.scalar.activation(o_t[:], ps_o[:], AF.Sigmoid)
                h_new = st.tile([128, GW], f16, tag="h16")
                nc.vector.tensor_mul(h_new[:], o_t[:], th[:])
                nc.sync.dma_start(outT_d[t], h_new[:])

                for it in drip_mm.get(t, ()):
                    drip_ps[it] = pa_mms(*it)
                for it in drip_ev.get(t, ()):
                    pa_evac(drip_ps.pop(it), *it)

                h16 = h_new
                c_st = c_new

            nc.sync.dma_start(cT_d[:], c_st[:])

    nc.finalize()
    return nc


def _prep_weights(W_list):
    # W [H, K] -> tiles [(g qo qi) p_in, p_out]
    out = []
    for W in W_list:
        Wt = np.asarray(W, np.float32).reshape(QH, 128, -1, 128)  # qo p_out qi p_in
        out.append(np.transpose(Wt, (0, 2, 3, 1)))  # qo qi p_in p_out
    arr = np.stack(out, 0)  # g qo qi p_in p_out
    return np.ascontiguousarray(arr.reshape(-1, 128)).astype(np.float16)


def kernel(x, W_xi, W_hi, b_i, W_xc, W_hc, b_c, W_xo, W_ho, b_o):
    x = np.asarray(x, np.float32)
    # gate order (c, i, o)
    WxT = _prep_weights([W_xc, W_xi, W_xo])
    WhT = _prep_weights([W_hc, W_hi, W_ho])
    bias = np.stack(
        [np.asarray(b, np.float32).reshape(QH, 128).T for b in (b_c, b_i, b_o)], 1
    ).reshape(128, 3 * QH)

    if "nc" not in _CACHE:
        _CACHE["nc"] = _build_nc()
    nc = _CACHE["nc"]

    in_maps = []
    for c in range(NCORES):
        xs = x[:, c * BL : (c + 1) * BL, :]  # [S, BL, I]
        xT = np.ascontiguousarray(np.transpose(xs, (2, 0, 1)).reshape(I, S * BL))
        in_maps.append(
            {
                "xT": xT.astype(np.float16),
                "WxT": WxT,
                "WhT": WhT,
                "bias": bias,
                "ident": np.eye(128, dtype=np.float16),
            }
        )

    from concourse.bass_utils import run_bass_kernel_spmd

    _CACHE["in_maps"] = in_maps
    res = run_bass_kernel_spmd(nc, in_maps, list(range(NCORES)))

    output = np.empty((S, B, H), np.float32)
    c_fin = np.empty((B, H), np.float32)
    for c in range(NCORES):
        oT = res.results[c]["outT"].astype(np.float32)  # [S, 128, QH*BL]
        output[:, c * BL : (c + 1) * BL, :] = (
            oT.reshape(S, 128, QH, BL).transpose(0, 3, 2, 1).reshape(S, BL, H)
        )
        cT = res.results[c]["cT"]
        c_fin[c * BL : (c + 1) * BL] = (
            cT.reshape(128, QH, BL).transpose(2, 1, 0).reshape(BL, H)
        )
    h_fin = np.ascontiguousarray(output[-1])
    return output, h_fin, c_fin
